# revision 9
# baseline (speedup 1.0000x reference)
"""AAST block kernel for 8 TRN2 NeuronCores.

Sharding: data-parallel over batch B=16 -> 2 batches per core, all params
replicated. No collectives. Device computes the dominant FLOPs (spatial
attention chain: prod2 -> sigmoid -> S=Vs@sig -> softmax -> threshold -> spat,
and the K-order graph conv h_k = A_k^T @ x -> Theta -> relu -> gcn) in
float32r matmuls (full-rate fp32-storage matmul mode, ~1e-3 rel precision).
Tiny temporal attention (T=12 matrices), the N x N support polynomials, and
the cheap depthwise time-conv + layernorm epilogue run on host numpy.
"""
import numpy as np

import concourse.bass as bass
import concourse.tile as tile
from concourse import bacc, mybir
from concourse.bass_utils import run_bass_kernel_spmd

B, N, F, T = 16, 1024, 32, 12
K, FC, FT, TS = 3, 64, 64, 1
THETA_THR = 0.6
LN_EPS = 1e-5
NCORES = 8
BLOC = B // NCORES  # 2 batches per core

f32 = mybir.dt.float32
f32r = mybir.dt.float32r

NT = N // 128  # 8 n-tiles
JT = (T * F) // 128  # 3 tiles of the (t,f) axis

_CACHE = {}


def _build():
    nc = bacc.Bacc("TRN2", target_bir_lowering=False, debug=False,
                   num_devices=NCORES)

    # per-core inputs ------------------------------------------------------
    xtf = nc.declare_dram_parameter("xtf", [BLOC, N, T * F], f32, isOutput=False)
    l2T = nc.declare_dram_parameter("l2T", [BLOC, T, N], f32, isOutput=False)
    r2d = nc.declare_dram_parameter("r2", [BLOC, T, N], f32, isOutput=False)
    bsd = nc.declare_dram_parameter("bs", [N, N], f32, isOutput=False)
    vsT = nc.declare_dram_parameter("vsT", [N, N], f32, isOutput=False)
    pol = nc.declare_dram_parameter("polys", [K, N, N], f32, isOutput=False)
    thd = nc.declare_dram_parameter("theta", [K, 128, FC], f32, isOutput=False)
    o_spat = nc.declare_dram_parameter("out_spat", [BLOC, N, N], f32, isOutput=True)
    # gcn transposed: [b, c, t, n]
    o_gcn = nc.declare_dram_parameter("out_gcn", [BLOC, FC, T, N], f32, isOutput=True)
    scr = nc.dram_tensor("thr_scratch", [BLOC, 1], f32)

    with tile.TileContext(nc) as tc:
        with (
            tc.tile_pool(name="const", bufs=1) as cpool,
            tc.tile_pool(name="stage", bufs=2) as spool,
            tc.tile_pool(name="mats", bufs=1) as mpool,
            tc.tile_pool(name="work", bufs=1) as wpool,
            tc.tile_pool(name="psA", bufs=3, space="PSUM") as psA,
            tc.tile_pool(name="psS", bufs=1, space="PSUM") as psS,
            tc.tile_pool(name="psG", bufs=2, space="PSUM") as psG,
        ):
            # ---- constants resident across both batches ----
            ones_f = cpool.tile([128, 1], f32, tag="onesf")
            nc.vector.memset(ones_f, 1.0)
            ones128 = cpool.tile([128, 1], f32r, tag="ones")
            nc.vector.tensor_copy(ones128, ones_f)

            vsr = []  # VsT rounded to f32r, tiles [128, N] (rows m-tile)
            for mt in range(NT):
                stg = spool.tile([128, N], f32, tag="bstage")
                nc.sync.dma_start(out=stg, in_=vsT[mt * 128:(mt + 1) * 128, :])
                t_ = mpool.tile([128, N], f32r, tag=f"vsr{mt}")
                nc.vector.tensor_copy(t_, stg)
                vsr.append(t_)

            thr_t = []  # Theta rounded, [F, FC] each
            for k in range(K):
                stg = spool.tile([128, FC], f32, tag="thstage")
                nc.sync.dma_start(out=stg, in_=thd[k])
                t_ = cpool.tile([128, FC], mybir.dt.bfloat16, tag=f"theta{k}")
                nc.vector.tensor_copy(t_, stg)
                thr_t.append(t_)

            for b in range(BLOC):
                # ---- load per-batch smalls ----
                l2r = wpool.tile([T, N], f32r, tag="l2r")
                r2r = wpool.tile([T, N], f32r, tag="r2r")
                stg1 = spool.tile([T, N], f32, tag="tstage")
                nc.sync.dma_start(out=stg1, in_=l2T[b])
                nc.vector.tensor_copy(l2r, stg1)
                stg2 = spool.tile([T, N], f32, tag="tstage")
                nc.sync.dma_start(out=stg2, in_=r2d[b])
                nc.vector.tensor_copy(r2r, stg2)

                xr = []  # x[b] as [n, (t,f)] rounded
                for nt in range(NT):
                    stg = spool.tile([128, T * F], f32, tag="xstage")
                    nc.sync.dma_start(out=stg, in_=xtf[b, nt * 128:(nt + 1) * 128, :])
                    t_ = wpool.tile([128, T * F], f32r, tag=f"xr{nt}")
                    nc.vector.tensor_copy(t_, stg)
                    xr.append(t_)

                # ---- prod2 -> sigmoid(prod2 + bs) ----
                sig = [wpool.tile([128, N], f32r, tag=f"sig{nt}", name=f"sig{nt}") for nt in range(NT)]
                for nt in range(NT):
                    bst = spool.tile([128, N], f32, tag="bstage")
                    nc.sync.dma_start(out=bst, in_=bsd[nt * 128:(nt + 1) * 128, :])
                    for mh in range(2):
                        pp = psA.tile([128, 512], f32, tag="pp")
                        nc.tensor.matmul(
                            pp, l2r[:, nt * 128:(nt + 1) * 128],
                            r2r[:, mh * 512:(mh + 1) * 512],
                            start=True, stop=True)
                        sl = sig[nt][:, mh * 512:(mh + 1) * 512]
                        nc.vector.tensor_add(sl, pp, bst[:, mh * 512:(mh + 1) * 512])
                        nc.scalar.activation(
                            sl, sl, mybir.ActivationFunctionType.Sigmoid)

                # ---- S = Vs @ sig  (via lhsT = VsT tiles), exp ----
                expS = [wpool.tile([128, N], f32r, tag=f"expS{nt}", name=f"expS{nt}_{b}") for nt in range(NT)]
                for nt in range(NT):
                    for kh in range(2):
                        ps = psA.tile([128, 512], f32, tag="pp")
                        for mt in range(NT):
                            nc.tensor.matmul(
                                ps, vsr[mt][:, nt * 128:(nt + 1) * 128],
                                sig[mt][:, kh * 512:(kh + 1) * 512],
                                start=(mt == 0), stop=(mt == NT - 1))
                        nc.scalar.activation(
                            expS[nt][:, kh * 512:(kh + 1) * 512], ps,
                            mybir.ActivationFunctionType.Exp)

                # ---- column sums of expS -> reciprocal -> thr ----
                cs = spool.tile([1, N], f32, tag="cs")
                for kh in range(2):
                    pc = psS.tile([1, 512], f32, tag="pcs")
                    for nt in range(NT):
                        nc.tensor.matmul(
                            pc, ones128, expS[nt][:, kh * 512:(kh + 1) * 512],
                            start=(nt == 0), stop=(nt == NT - 1))
                    nc.vector.tensor_copy(cs[:, kh * 512:(kh + 1) * 512], pc)
                rcs = spool.tile([1, N], f32, tag="rcs")
                nc.vector.reciprocal(rcs, cs)
                # total sum of Sn = sum_k cs[k]*rcs[k] ( = N up to fp error)
                prod_cr = spool.tile([1, N], f32, tag="pcr")
                nc.vector.tensor_mul(prod_cr, cs, rcs)
                tot = spool.tile([1, 1], f32, tag="tot")
                nc.vector.reduce_sum(tot, prod_cr, axis=mybir.AxisListType.X)
                thr1 = spool.tile([1, 1], f32, tag="thr1")
                nc.scalar.mul(thr1, tot, 1.0 / (float(N) * float(N) * THETA_THR))
                nc.sync.dma_start(out=scr[b], in_=thr1)
                thr128 = spool.tile([128, 1], f32, tag="thr128")
                nc.sync.dma_start(
                    out=thr128,
                    in_=bass.AP(tensor=scr.ap().tensor,
                                offset=scr.ap().offset + b,
                                ap=[[0, 128], [1, 1]]))
                # broadcast rcs to all partitions via DRAM bounce
                rcs_d = nc.dram_tensor(f"rcs_d{b}", [N], f32)
                nc.sync.dma_start(out=rcs_d[:], in_=rcs)
                rcs_b = spool.tile([128, N], f32, tag="rcsb", bufs=1)
                nc.sync.dma_start(
                    out=rcs_b,
                    in_=bass.AP(tensor=rcs_d.ap().tensor,
                                offset=rcs_d.ap().offset,
                                ap=[[0, 128], [1, N]]))

                # ---- Sn, threshold -> spat (fp32 out + f32r for conv) ----
                spr = expS
                for nt in range(NT):
                    sn = spool.tile([128, N], f32, tag="sn", bufs=1)
                    nc.vector.tensor_mul(sn, expS[nt], rcs_b)
                    mask = spool.tile([128, N], f32, tag="mask", bufs=1)
                    nc.vector.tensor_scalar(
                        mask, sn, thr128, None, op0=mybir.AluOpType.is_ge)
                    nc.vector.tensor_mul(sn, sn, mask)
                    nc.sync.dma_start(
                        out=o_spat[b, nt * 128:(nt + 1) * 128, :], in_=sn)
                    nc.vector.tensor_copy(spr[nt], sn)

                # ---- graph conv: hT_k = x^T @ (polys_k * spat) ----
                hsb = {}
                for k in range(K):
                    ak = [wpool.tile([128, N], f32r, tag=f"sig{nt}", name=f"ak{nt}_{b}_{k}") for nt in range(NT)]
                    for nt in range(NT):
                        pst = spool.tile([128, N], f32, tag="pstage")
                        nc.sync.dma_start(
                            out=pst, in_=pol[k, nt * 128:(nt + 1) * 128, :])
                        nc.vector.tensor_mul(ak[nt], pst, spr[nt])
                    for jt in range(JT):
                        hs = wpool.tile([128, N], mybir.dt.bfloat16, tag=f"hsb{k}_{jt}")
                        hsb[(k, jt)] = hs
                        for mh in range(2):
                            ph = psA.tile([128, 512], f32, tag="pp")
                            for nt in range(NT):
                                nc.tensor.matmul(
                                    ph, xr[nt][:, jt * 128:(jt + 1) * 128],
                                    ak[nt][:, mh * 512:(mh + 1) * 512],
                                    start=(nt == 0), stop=(nt == NT - 1))
                            nc.vector.tensor_copy(
                                hs[:, mh * 512:(mh + 1) * 512], ph)

                # ---- out[t] = relu( sum_k hT_k[t-rows].T @ Theta_k ) ----
                for mh in range(2):
                    for t in range(T):
                        jt, row = divmod(t * F, 128)
                        pg = psG.tile([FC, 512], f32, tag="pg")
                        for k in range(K):
                            nc.tensor.matmul(
                                pg, thr_t[k][row:row + F, :],
                                hsb[(k, jt)][row:row + F, mh * 512:(mh + 1) * 512],
                                start=(k == 0), stop=(k == K - 1),
                                tile_position=(row, 0))
                        gch = spool.tile([FC, 512], f32, tag="gch", bufs=3)
                        nc.scalar.activation(
                            gch, pg, mybir.ActivationFunctionType.Relu)
                        nc.sync.dma_start(
                            out=o_gcn[b, :, t, mh * 512:(mh + 1) * 512], in_=gch)

    nc.compile()
    return nc


def _host_pre(x, node_embeddings, W1, W2, W3, U1, U2, U3, be, Ve):
    """Temporal attention (tiny) + supports, in float64 host math."""
    xd = x.astype(np.float64)
    # E (B,T,T)
    lhs = np.einsum('bnft,n->btf', xd, U1.astype(np.float64)) @ U2.astype(np.float64)
    rhs = np.einsum('f,bnft->bnt', U3.astype(np.float64), xd)
    prod = np.einsum('btn,bnu->btu', lhs, rhs)
    sig = 1.0 / (1.0 + np.exp(-(prod + be.astype(np.float64))))
    E = np.einsum('ts,bsu->btu', Ve.astype(np.float64), sig)
    E = E - E.max(axis=1, keepdims=True)
    E = np.exp(E)
    E = E / E.sum(axis=1, keepdims=True)
    # fold x_TAt away:  l2 = (x . (E @ W1)) @ W2 ;  r2 = E^T @ (W3 . x)
    e1 = E @ W1.astype(np.float64)                      # (B,T)
    l2a = np.einsum('bnfs,bs->bnf', xd, e1)             # (B,N,F)
    l2 = l2a @ W2.astype(np.float64)                    # (B,N,T)
    w3x = np.einsum('f,bmfs->bsm', W3.astype(np.float64), xd)   # (B,T,N)
    r2 = np.einsum('bst,bsm->btm', E, w3x)              # (B,T,N)
    l2T = np.ascontiguousarray(l2.transpose(0, 2, 1))   # (B,T,N)
    # supports
    emb = node_embeddings.astype(np.float64)
    g = emb @ emb.T
    g = np.maximum(g, 0.0)
    g = np.exp(g - g.max(axis=1, keepdims=True))
    sup = g / g.sum(axis=1, keepdims=True)
    polys = np.stack([np.eye(N), sup, 2.0 * (sup @ sup) - np.eye(N)])
    return (l2T.astype(np.float32), r2.astype(np.float32),
            polys.astype(np.float32))


def _host_post(x, gcn, tc_v, tc_g, tc_b, rc_w, rc_b, ln_w, ln_b):
    """time conv (weight-normed 1x3) + residual 1x1 conv + layernorm."""
    xd = x.astype(np.float64)
    g = gcn.astype(np.float64)                          # (B,T,N,FC)
    vnorm = np.sqrt((tc_v.astype(np.float64) ** 2).sum(axis=(1, 2, 3), keepdims=True))
    w = tc_g.astype(np.float64)[:, None, None, None] * tc_v.astype(np.float64) / vnorm
    w = w[:, :, 0, :]                                   # (FT, FC, 3)
    gp = np.pad(g, ((0, 0), (1, 1), (0, 0), (0, 0)))    # pad t
    # tco[b,o,n,t] = sum_c sum_d w[o,c,d] * gp[b,t+d,n,c]
    tco = np.einsum('ocd,bdtnc->bont',
                    w, np.stack([gp[:, d:d + T] for d in range(3)], axis=1))
    tco = tco + tc_b.astype(np.float64)[None, :, None, None]
    res = np.einsum('of,bnft->bont', rc_w.astype(np.float64)[:, :, 0, 0], xd)
    res = res + rc_b.astype(np.float64)[None, :, None, None]
    y = (res + tco).transpose(0, 3, 2, 1)               # (B,T,N,FT)
    mu = y.mean(axis=-1, keepdims=True)
    var = y.var(axis=-1, keepdims=True)
    y = (y - mu) / np.sqrt(var + LN_EPS) * ln_w.astype(np.float64) + ln_b.astype(np.float64)
    return y.transpose(0, 2, 3, 1).astype(np.float32)   # (B,N,FT,T)


def kernel(x, node_embeddings, W1, W2, W3, bs, Vs, U1, U2, U3, be, Ve,
           Theta, tc_v, tc_g, tc_b, rc_w, rc_b, ln_w, ln_b):
    x = np.asarray(x, dtype=np.float32)
    l2T, r2, polys = _host_pre(
        x, np.asarray(node_embeddings), np.asarray(W1), np.asarray(W2),
        np.asarray(W3), np.asarray(U1), np.asarray(U2), np.asarray(U3),
        np.asarray(be), np.asarray(Ve))

    xtf = np.ascontiguousarray(x.transpose(0, 1, 3, 2).reshape(B, N, T * F))
    vsT = np.ascontiguousarray(np.asarray(Vs, dtype=np.float32).T)
    bs2 = np.ascontiguousarray(np.asarray(bs, dtype=np.float32)[0])
    theta = np.ascontiguousarray(np.tile(np.asarray(Theta, dtype=np.float32), (1, 4, 1)))

    if "nc" not in _CACHE:
        _CACHE["nc"] = _build()
    nc = _CACHE["nc"]

    in_maps = []
    for c in range(NCORES):
        sl = slice(c * BLOC, (c + 1) * BLOC)
        in_maps.append(dict(
            xtf=xtf[sl], l2T=l2T[sl], r2=r2[sl], bs=bs2, vsT=vsT,
            polys=polys, theta=theta))
    res = run_bass_kernel_spmd(nc, in_maps, core_ids=list(range(NCORES)))

    spat = np.concatenate([r["out_spat"] for r in res.results], axis=0)
    gcnT = np.concatenate([r["out_gcn"] for r in res.results], axis=0)
    gcn = gcnT.transpose(0, 2, 3, 1)                    # (B,T,N,FC)

    x_res = _host_post(x, gcn, np.asarray(tc_v), np.asarray(tc_g),
                       np.asarray(tc_b), np.asarray(rc_w), np.asarray(rc_b),
                       np.asarray(ln_w), np.asarray(ln_b))
    return x_res, spat


# revision 10
# speedup vs baseline: 1.8910x; 1.8910x over previous
"""AAST block kernel for 8 TRN2 NeuronCores.

Sharding: data-parallel over batch B=16 -> 2 batches per core, all params
replicated. No collectives. Device computes the dominant FLOPs (spatial
attention chain: prod2 -> sigmoid -> S=Vs@sig -> softmax -> threshold -> spat,
and the K-order graph conv h_k = A_k^T @ x -> Theta -> relu -> gcn) in
float32r matmuls (full-rate fp32-storage matmul mode, ~1e-3 rel precision).
Tiny temporal attention (T=12 matrices), the N x N support polynomials, and
the cheap depthwise time-conv + layernorm epilogue run on host numpy.
"""
import numpy as np

import concourse.bass as bass
import concourse.tile as tile
from concourse import bacc, mybir
from concourse.bass_utils import run_bass_kernel_spmd

B, N, F, T = 16, 1024, 32, 12
K, FC, FT, TS = 3, 64, 64, 1
THETA_THR = 0.6
LN_EPS = 1e-5
NCORES = 8
BLOC = B // NCORES  # 2 batches per core

f32 = mybir.dt.float32
f32r = mybir.dt.float32r

NT = N // 128  # 8 n-tiles
JT = (T * F) // 128  # 3 tiles of the (t,f) axis

_CACHE = {}


def _build():
    nc = bacc.Bacc("TRN2", target_bir_lowering=False, debug=False,
                   num_devices=NCORES)

    # per-core inputs ------------------------------------------------------
    xtf = nc.declare_dram_parameter("xtf", [BLOC, N, T * F], f32, isOutput=False)
    l2T = nc.declare_dram_parameter("l2T", [BLOC, T, N], f32, isOutput=False)
    r2d = nc.declare_dram_parameter("r2", [BLOC, T, N], f32, isOutput=False)
    bsd = nc.declare_dram_parameter("bs", [N, N], f32, isOutput=False)
    vsT = nc.declare_dram_parameter("vsT", [N, N], f32, isOutput=False)
    pol = nc.declare_dram_parameter("polys", [K, N, N], f32, isOutput=False)
    thd = nc.declare_dram_parameter("theta", [K, 128, FC], f32, isOutput=False)
    o_spat = nc.declare_dram_parameter("out_spat", [BLOC, N, N], f32, isOutput=True)
    # gcn transposed: [b, c, t, n]
    o_gcn = nc.declare_dram_parameter("out_gcn", [BLOC, FC, T, N], f32, isOutput=True)
    scr = nc.dram_tensor("thr_scratch", [BLOC, 1], f32)

    with tile.TileContext(nc) as tc:
        with (
            tc.tile_pool(name="const", bufs=1) as cpool,
            tc.tile_pool(name="stage", bufs=2) as spool,
            tc.tile_pool(name="mats", bufs=1) as mpool,
            tc.tile_pool(name="work", bufs=1) as wpool,
            tc.tile_pool(name="psA", bufs=3, space="PSUM") as psA,
            tc.tile_pool(name="psS", bufs=1, space="PSUM") as psS,
            tc.tile_pool(name="psG", bufs=2, space="PSUM") as psG,
        ):
            # ---- constants resident across both batches ----
            ones_f = cpool.tile([128, 1], f32, tag="onesf")
            nc.vector.memset(ones_f, 1.0)
            ones128 = cpool.tile([128, 1], f32r, tag="ones")
            nc.vector.tensor_copy(ones128, ones_f)

            vsr = []  # VsT rounded to f32r, tiles [128, N] (rows m-tile)
            for mt in range(NT):
                stg = spool.tile([128, N], f32, tag="bstage")
                nc.sync.dma_start(out=stg, in_=vsT[mt * 128:(mt + 1) * 128, :])
                t_ = mpool.tile([128, N], f32r, tag=f"vsr{mt}")
                nc.vector.tensor_copy(t_, stg)
                vsr.append(t_)

            thr_t = []  # Theta rounded, [F, FC] each
            for k in range(K):
                stg = spool.tile([128, FC], f32, tag="thstage")
                nc.sync.dma_start(out=stg, in_=thd[k])
                t_ = cpool.tile([128, FC], mybir.dt.bfloat16, tag=f"theta{k}")
                nc.vector.tensor_copy(t_, stg)
                thr_t.append(t_)

            for b in range(BLOC):
                # ---- load per-batch smalls ----
                l2r = wpool.tile([T, N], f32r, tag="l2r")
                r2r = wpool.tile([T, N], f32r, tag="r2r")
                stg1 = spool.tile([T, N], f32, tag="tstage")
                nc.sync.dma_start(out=stg1, in_=l2T[b])
                nc.vector.tensor_copy(l2r, stg1)
                stg2 = spool.tile([T, N], f32, tag="tstage")
                nc.sync.dma_start(out=stg2, in_=r2d[b])
                nc.vector.tensor_copy(r2r, stg2)

                xr = []  # x[b] as [n, (t,f)] rounded
                for nt in range(NT):
                    stg = spool.tile([128, T * F], f32, tag="xstage")
                    nc.sync.dma_start(out=stg, in_=xtf[b, nt * 128:(nt + 1) * 128, :])
                    t_ = wpool.tile([128, T * F], f32r, tag=f"xr{nt}")
                    nc.vector.tensor_copy(t_, stg)
                    xr.append(t_)

                # ---- prod2 -> sigmoid(prod2 + bs) ----
                sig = [wpool.tile([128, N], f32r, tag=f"sig{nt}", name=f"sig{nt}") for nt in range(NT)]
                for nt in range(NT):
                    bst = spool.tile([128, N], f32, tag="bstage")
                    nc.sync.dma_start(out=bst, in_=bsd[nt * 128:(nt + 1) * 128, :])
                    for mh in range(2):
                        pp = psA.tile([128, 512], f32, tag="pp")
                        nc.tensor.matmul(
                            pp, l2r[:, nt * 128:(nt + 1) * 128],
                            r2r[:, mh * 512:(mh + 1) * 512],
                            start=True, stop=True)
                        sl = sig[nt][:, mh * 512:(mh + 1) * 512]
                        nc.vector.tensor_add(sl, pp, bst[:, mh * 512:(mh + 1) * 512])
                        nc.scalar.activation(
                            sl, sl, mybir.ActivationFunctionType.Sigmoid)

                # ---- S = Vs @ sig  (via lhsT = VsT tiles), exp ----
                expS = [wpool.tile([128, N], f32r, tag=f"expS{nt}", name=f"expS{nt}_{b}") for nt in range(NT)]
                for nt in range(NT):
                    for kh in range(2):
                        ps = psA.tile([128, 512], f32, tag="pp")
                        for mt in range(NT):
                            nc.tensor.matmul(
                                ps, vsr[mt][:, nt * 128:(nt + 1) * 128],
                                sig[mt][:, kh * 512:(kh + 1) * 512],
                                start=(mt == 0), stop=(mt == NT - 1))
                        nc.scalar.activation(
                            expS[nt][:, kh * 512:(kh + 1) * 512], ps,
                            mybir.ActivationFunctionType.Exp)

                # ---- column sums of expS -> reciprocal -> thr ----
                cs = spool.tile([1, N], f32, tag="cs")
                for kh in range(2):
                    pc = psS.tile([1, 512], f32, tag="pcs")
                    for nt in range(NT):
                        nc.tensor.matmul(
                            pc, ones128, expS[nt][:, kh * 512:(kh + 1) * 512],
                            start=(nt == 0), stop=(nt == NT - 1))
                    nc.vector.tensor_copy(cs[:, kh * 512:(kh + 1) * 512], pc)
                rcs = spool.tile([1, N], f32, tag="rcs")
                nc.vector.reciprocal(rcs, cs)
                # total sum of Sn = sum_k cs[k]*rcs[k] ( = N up to fp error)
                prod_cr = spool.tile([1, N], f32, tag="pcr")
                nc.vector.tensor_mul(prod_cr, cs, rcs)
                tot = spool.tile([1, 1], f32, tag="tot")
                nc.vector.reduce_sum(tot, prod_cr, axis=mybir.AxisListType.X)
                thr1 = spool.tile([1, 1], f32, tag="thr1")
                nc.scalar.mul(thr1, tot, 1.0 / (float(N) * float(N) * THETA_THR))
                nc.sync.dma_start(out=scr[b], in_=thr1)
                thr128 = spool.tile([128, 1], f32, tag="thr128")
                nc.sync.dma_start(
                    out=thr128,
                    in_=bass.AP(tensor=scr.ap().tensor,
                                offset=scr.ap().offset + b,
                                ap=[[0, 128], [1, 1]]))
                # broadcast rcs to all partitions via DRAM bounce
                rcs_d = nc.dram_tensor(f"rcs_d{b}", [N], f32)
                nc.sync.dma_start(out=rcs_d[:], in_=rcs)
                rcs_b = spool.tile([128, N], f32, tag="rcsb", bufs=1)
                nc.sync.dma_start(
                    out=rcs_b,
                    in_=bass.AP(tensor=rcs_d.ap().tensor,
                                offset=rcs_d.ap().offset,
                                ap=[[0, 128], [1, N]]))

                # ---- Sn, threshold -> spat (fp32 out + f32r for conv) ----
                spr = expS
                for nt in range(NT):
                    sn = spool.tile([128, N], f32, tag="sn", bufs=1)
                    nc.vector.tensor_mul(sn, expS[nt], rcs_b)
                    mask = spool.tile([128, N], f32, tag="mask", bufs=1)
                    nc.vector.tensor_scalar(
                        mask, sn, thr128, None, op0=mybir.AluOpType.is_ge)
                    nc.vector.tensor_mul(sn, sn, mask)
                    nc.sync.dma_start(
                        out=o_spat[b, nt * 128:(nt + 1) * 128, :], in_=sn)
                    nc.vector.tensor_copy(spr[nt], sn)

                # ---- graph conv: hT_k = x^T @ (polys_k * spat) ----
                hsb = {}
                for k in range(K):
                    ak = [wpool.tile([128, N], f32r, tag=f"sig{nt}", name=f"ak{nt}_{b}_{k}") for nt in range(NT)]
                    for nt in range(NT):
                        pst = spool.tile([128, N], f32, tag="pstage")
                        nc.sync.dma_start(
                            out=pst, in_=pol[k, nt * 128:(nt + 1) * 128, :])
                        nc.vector.tensor_mul(ak[nt], pst, spr[nt])
                    for jt in range(JT):
                        hs = wpool.tile([128, N], mybir.dt.bfloat16, tag=f"hsb{k}_{jt}")
                        hsb[(k, jt)] = hs
                        for mh in range(2):
                            ph = psA.tile([128, 512], f32, tag="pp")
                            for nt in range(NT):
                                nc.tensor.matmul(
                                    ph, xr[nt][:, jt * 128:(jt + 1) * 128],
                                    ak[nt][:, mh * 512:(mh + 1) * 512],
                                    start=(nt == 0), stop=(nt == NT - 1))
                            nc.vector.tensor_copy(
                                hs[:, mh * 512:(mh + 1) * 512], ph)

                # ---- out[t] = relu( sum_k hT_k[t-rows].T @ Theta_k ) ----
                for mh in range(2):
                    for t in range(T):
                        jt, row = divmod(t * F, 128)
                        pg = psG.tile([FC, 512], f32, tag="pg")
                        for k in range(K):
                            nc.tensor.matmul(
                                pg, thr_t[k][row:row + F, :],
                                hsb[(k, jt)][row:row + F, mh * 512:(mh + 1) * 512],
                                start=(k == 0), stop=(k == K - 1),
                                tile_position=(row, 0))
                        gch = spool.tile([FC, 512], f32, tag="gch", bufs=3)
                        nc.scalar.activation(
                            gch, pg, mybir.ActivationFunctionType.Relu)
                        nc.sync.dma_start(
                            out=o_gcn[b, :, t, mh * 512:(mh + 1) * 512], in_=gch)

    nc.compile()
    return nc


def _host_pre(x, node_embeddings, W1, W2, W3, U1, U2, U3, be, Ve):
    """Temporal attention (tiny) + supports, in float64 host math."""
    xd = x
    # E (B,T,T)
    lhs = np.einsum('bnft,n->btf', xd, U1, optimize=True) @ U2
    rhs = np.einsum('f,bnft->bnt', U3, xd, optimize=True)
    prod = np.einsum('btn,bnu->btu', lhs, rhs, optimize=True)
    sig = 1.0 / (1.0 + np.exp(-(prod + be)))
    E = np.einsum('ts,bsu->btu', Ve, sig, optimize=True)
    E = E - E.max(axis=1, keepdims=True)
    E = np.exp(E)
    E = E / E.sum(axis=1, keepdims=True)
    # fold x_TAt away:  l2 = (x . (E @ W1)) @ W2 ;  r2 = E^T @ (W3 . x)
    e1 = E @ W1                                         # (B,T)
    l2a = np.einsum('bnfs,bs->bnf', xd, e1, optimize=True)  # (B,N,F)
    l2 = l2a @ W2                                       # (B,N,T)
    w3x = np.einsum('f,bmfs->bsm', W3, xd, optimize=True)   # (B,T,N)
    r2 = np.einsum('bst,bsm->btm', E, w3x, optimize=True)   # (B,T,N)
    l2T = np.ascontiguousarray(l2.transpose(0, 2, 1))   # (B,T,N)
    # supports
    emb = node_embeddings
    g = emb @ emb.T
    g = np.maximum(g, 0.0)
    g = np.exp(g - g.max(axis=1, keepdims=True))
    sup = g / g.sum(axis=1, keepdims=True)
    eye = np.eye(N, dtype=np.float32)
    polys = np.stack([eye, sup, 2.0 * (sup @ sup) - eye])
    return (np.ascontiguousarray(l2T, dtype=np.float32),
            np.ascontiguousarray(r2, dtype=np.float32),
            np.ascontiguousarray(polys, dtype=np.float32))


def _host_post(x, gcn, tc_v, tc_g, tc_b, rc_w, rc_b, ln_w, ln_b):
    """time conv (weight-normed 1x3) + residual 1x1 conv + layernorm."""
    g = gcn                                             # (B,T,N,FC)
    vnorm = np.sqrt((tc_v.astype(np.float64) ** 2).sum(axis=(1, 2, 3), keepdims=True))
    w = (tc_g[:, None, None, None] * tc_v / vnorm.astype(np.float32))[:, :, 0, :]
    gp = np.pad(g, ((0, 0), (1, 1), (0, 0), (0, 0)))    # pad t -> (B,T+2,N,C)
    # y built directly in (B,T,N,FT): yt[b,t,n,o]
    yt = np.einsum('btnc,oc->btno', gp[:, 0:T], w[:, :, 0], optimize=True)
    yt += np.einsum('btnc,oc->btno', gp[:, 1:T + 1], w[:, :, 1], optimize=True)
    yt += np.einsum('btnc,oc->btno', gp[:, 2:T + 2], w[:, :, 2], optimize=True)
    yt += tc_b[None, None, None, :]
    yt += np.einsum('bnft,of->btno', x, rc_w[:, :, 0, 0], optimize=True)
    yt += rc_b[None, None, None, :]
    mu = yt.mean(axis=-1, keepdims=True)
    var = yt.var(axis=-1, keepdims=True)
    yt = (yt - mu) / np.sqrt(var + LN_EPS) * ln_w + ln_b
    return np.ascontiguousarray(yt.transpose(0, 2, 3, 1)).astype(np.float32)


def kernel(x, node_embeddings, W1, W2, W3, bs, Vs, U1, U2, U3, be, Ve,
           Theta, tc_v, tc_g, tc_b, rc_w, rc_b, ln_w, ln_b):
    x = np.asarray(x, dtype=np.float32)
    l2T, r2, polys = _host_pre(
        x, np.asarray(node_embeddings), np.asarray(W1), np.asarray(W2),
        np.asarray(W3), np.asarray(U1), np.asarray(U2), np.asarray(U3),
        np.asarray(be), np.asarray(Ve))

    xtf = np.ascontiguousarray(x.transpose(0, 1, 3, 2).reshape(B, N, T * F))
    vsT = np.ascontiguousarray(np.asarray(Vs, dtype=np.float32).T)
    bs2 = np.ascontiguousarray(np.asarray(bs, dtype=np.float32)[0])
    theta = np.ascontiguousarray(np.tile(np.asarray(Theta, dtype=np.float32), (1, 4, 1)))

    if "nc" not in _CACHE:
        _CACHE["nc"] = _build()
    nc = _CACHE["nc"]

    in_maps = []
    for c in range(NCORES):
        sl = slice(c * BLOC, (c + 1) * BLOC)
        in_maps.append(dict(
            xtf=xtf[sl], l2T=l2T[sl], r2=r2[sl], bs=bs2, vsT=vsT,
            polys=polys, theta=theta))
    res = run_bass_kernel_spmd(nc, in_maps, core_ids=list(range(NCORES)))

    spat = np.concatenate([r["out_spat"] for r in res.results], axis=0)
    gcnT = np.concatenate([r["out_gcn"] for r in res.results], axis=0)
    gcn = gcnT.transpose(0, 2, 3, 1)                    # (B,T,N,FC)

    x_res = _host_post(x, gcn, np.asarray(tc_v), np.asarray(tc_g),
                       np.asarray(tc_b), np.asarray(rc_w), np.asarray(rc_b),
                       np.asarray(ln_w), np.asarray(ln_b))
    return x_res, spat


# revision 11
# speedup vs baseline: 2.2305x; 1.1795x over previous
"""AAST block kernel for 8 TRN2 NeuronCores.

Sharding: data-parallel over batch B=16 -> 2 batches per core, all params
replicated. No collectives. Device computes the dominant FLOPs (spatial
attention chain: prod2 -> sigmoid -> S=Vs@sig -> softmax -> threshold -> spat,
and the K-order graph conv h_k = A_k^T @ x -> Theta -> relu -> gcn) in
float32r matmuls (full-rate fp32-storage matmul mode, ~1e-3 rel precision).
Tiny temporal attention (T=12 matrices), the N x N support polynomials, and
the cheap depthwise time-conv + layernorm epilogue run on host numpy.
"""
import numpy as np

import concourse.bass as bass
import concourse.tile as tile
from concourse import bacc, mybir
from concourse.bass_utils import run_bass_kernel_spmd

B, N, F, T = 16, 1024, 32, 12
K, FC, FT, TS = 3, 64, 64, 1
THETA_THR = 0.6
LN_EPS = 1e-5
NCORES = 8
BLOC = B // NCORES  # 2 batches per core

f32 = mybir.dt.float32
f32r = mybir.dt.float32r

NT = N // 128  # 8 n-tiles
JT = (T * F) // 128  # 3 tiles of the (t,f) axis

_CACHE = {}


def _build():
    nc = bacc.Bacc("TRN2", target_bir_lowering=False, debug=False,
                   num_devices=NCORES)

    # per-core inputs ------------------------------------------------------
    xtf = nc.declare_dram_parameter("xtf", [BLOC, N, T * F], f32r, isOutput=False)
    l2T = nc.declare_dram_parameter("l2T", [BLOC, T, N], f32r, isOutput=False)
    r2d = nc.declare_dram_parameter("r2", [BLOC, T, N], f32r, isOutput=False)
    bsd = nc.declare_dram_parameter("bs", [N, N], f32r, isOutput=False)
    vsT = nc.declare_dram_parameter("vsT", [N, N], f32r, isOutput=False)
    pol = nc.declare_dram_parameter("polys", [K, N, N], f32r, isOutput=False)
    thd = nc.declare_dram_parameter("theta", [K, 128, FC], f32, isOutput=False)
    eyed = nc.declare_dram_parameter("eye", [128, 128], f32r, isOutput=False)
    o_spat = nc.declare_dram_parameter("out_spat", [BLOC, N, N], f32, isOutput=True)
    # gcn transposed: [b, c, t, n]
    o_gcn = nc.declare_dram_parameter("out_gcn", [BLOC, FC, T, N], f32, isOutput=True)
    scr = nc.dram_tensor("thr_scratch", [BLOC, 1], f32)

    with tile.TileContext(nc) as tc:
        with (
            tc.tile_pool(name="const", bufs=1) as cpool,
            tc.tile_pool(name="stage", bufs=2) as spool,
            tc.tile_pool(name="mats", bufs=1) as mpool,
            tc.tile_pool(name="work", bufs=1) as wpool,
            tc.tile_pool(name="psA", bufs=3, space="PSUM") as psA,
            tc.tile_pool(name="psS", bufs=1, space="PSUM") as psS,
            tc.tile_pool(name="psG", bufs=2, space="PSUM") as psG,
        ):
            # ---- constants resident across both batches ----
            ones_f = cpool.tile([128, 1], f32, tag="onesf")
            nc.vector.memset(ones_f, 1.0)
            ones128 = cpool.tile([128, 1], f32r, tag="ones")
            nc.vector.tensor_copy(ones128, ones_f)

            vsr = []  # VsT tiles [128, N] (rows m-tile), f32r direct
            for mt in range(NT):
                t_ = mpool.tile([128, N], f32r, tag=f"vsr{mt}")
                nc.sync.dma_start(out=t_, in_=vsT[mt * 128:(mt + 1) * 128, :])
                vsr.append(t_)
            eyer = cpool.tile([128, 128], f32r, tag="eyer")
            nc.sync.dma_start(out=eyer, in_=eyed[:, :])

            thr_t = []  # Theta rounded, [F, FC] each
            for k in range(K):
                stg = spool.tile([128, FC], f32, tag="thstage")
                nc.sync.dma_start(out=stg, in_=thd[k])
                t_ = cpool.tile([128, FC], mybir.dt.bfloat16, tag=f"theta{k}")
                nc.vector.tensor_copy(t_, stg)
                thr_t.append(t_)

            for b in range(BLOC):
                # ---- load per-batch smalls ----
                l2r = wpool.tile([T, N], f32r, tag="l2r")
                r2r = wpool.tile([T, N], f32r, tag="r2r")
                nc.sync.dma_start(out=l2r, in_=l2T[b])
                nc.sync.dma_start(out=r2r, in_=r2d[b])

                xr = []  # x[b] as [n, (t,f)], f32r direct
                for nt in range(NT):
                    t_ = wpool.tile([128, T * F], f32r, tag=f"xr{nt}")
                    nc.sync.dma_start(out=t_, in_=xtf[b, nt * 128:(nt + 1) * 128, :])
                    xr.append(t_)

                # ---- prod2 -> sigmoid(prod2 + bs) ----
                sig = [wpool.tile([128, N], f32r, tag=f"sig{nt}", name=f"sig{nt}") for nt in range(NT)]
                for nt in range(NT):
                    bst = spool.tile([128, N], f32r, tag="bstage")
                    nc.sync.dma_start(out=bst, in_=bsd[nt * 128:(nt + 1) * 128, :])
                    for mh in range(2):
                        pp = psA.tile([128, 512], f32, tag="pp")
                        nc.tensor.matmul(
                            pp, l2r[:, nt * 128:(nt + 1) * 128],
                            r2r[:, mh * 512:(mh + 1) * 512],
                            start=True, stop=False)
                        nc.tensor.matmul(
                            pp, eyer, bst[:, mh * 512:(mh + 1) * 512],
                            start=False, stop=True)
                        nc.scalar.activation(
                            sig[nt][:, mh * 512:(mh + 1) * 512], pp,
                            mybir.ActivationFunctionType.Sigmoid)

                # ---- S = Vs @ sig  (via lhsT = VsT tiles), exp ----
                expS = [wpool.tile([128, N], f32r, tag=f"expS{nt}", name=f"expS{nt}_{b}") for nt in range(NT)]
                for nt in range(NT):
                    for kh in range(2):
                        ps = psA.tile([128, 512], f32, tag="pp")
                        for mt in range(NT):
                            nc.tensor.matmul(
                                ps, vsr[mt][:, nt * 128:(nt + 1) * 128],
                                sig[mt][:, kh * 512:(kh + 1) * 512],
                                start=(mt == 0), stop=(mt == NT - 1))
                        nc.scalar.activation(
                            expS[nt][:, kh * 512:(kh + 1) * 512], ps,
                            mybir.ActivationFunctionType.Exp)

                # ---- column sums of expS -> reciprocal -> thr ----
                cs = spool.tile([1, N], f32, tag="cs")
                for kh in range(2):
                    pc = psS.tile([1, 512], f32, tag="pcs")
                    for nt in range(NT):
                        nc.tensor.matmul(
                            pc, ones128, expS[nt][:, kh * 512:(kh + 1) * 512],
                            start=(nt == 0), stop=(nt == NT - 1))
                    nc.vector.tensor_copy(cs[:, kh * 512:(kh + 1) * 512], pc)
                rcs = spool.tile([1, N], f32, tag="rcs")
                nc.vector.reciprocal(rcs, cs)
                # total sum of Sn = sum_k cs[k]*rcs[k] ( = N up to fp error)
                prod_cr = spool.tile([1, N], f32, tag="pcr")
                nc.vector.tensor_mul(prod_cr, cs, rcs)
                tot = spool.tile([1, 1], f32, tag="tot")
                nc.vector.reduce_sum(tot, prod_cr, axis=mybir.AxisListType.X)
                thr1 = spool.tile([1, 1], f32, tag="thr1")
                nc.scalar.mul(thr1, tot, 1.0 / (float(N) * float(N) * THETA_THR))
                nc.sync.dma_start(out=scr[b], in_=thr1)
                thr128 = spool.tile([128, 1], f32, tag="thr128")
                nc.sync.dma_start(
                    out=thr128,
                    in_=bass.AP(tensor=scr.ap().tensor,
                                offset=scr.ap().offset + b,
                                ap=[[0, 128], [1, 1]]))
                # broadcast rcs to all partitions via DRAM bounce
                rcs_d = nc.dram_tensor(f"rcs_d{b}", [N], f32)
                nc.sync.dma_start(out=rcs_d[:], in_=rcs)
                rcs_b = spool.tile([128, N], f32, tag="rcsb", bufs=1)
                nc.sync.dma_start(
                    out=rcs_b,
                    in_=bass.AP(tensor=rcs_d.ap().tensor,
                                offset=rcs_d.ap().offset,
                                ap=[[0, 128], [1, N]]))

                # ---- Sn, threshold -> spat (fp32 out + f32r for conv) ----
                spr = expS
                for nt in range(NT):
                    sn = spool.tile([128, N], f32, tag="sn", bufs=1)
                    nc.vector.tensor_mul(sn, expS[nt], rcs_b)
                    mask = spool.tile([128, N], f32, tag="mask", bufs=1)
                    nc.vector.tensor_scalar(
                        mask, sn, thr128, None, op0=mybir.AluOpType.is_ge)
                    nc.vector.tensor_mul(spr[nt], sn, mask)
                    nc.sync.dma_start(
                        out=o_spat[b, nt * 128:(nt + 1) * 128, :],
                        in_=spr[nt].bitcast(f32))

                # ---- graph conv: hT_k = x^T @ (polys_k * spat) ----
                hsb = {}
                for k in range(K):
                    ak = [wpool.tile([128, N], f32r, tag=f"sig{nt}", name=f"ak{nt}_{b}_{k}") for nt in range(NT)]
                    for nt in range(NT):
                        pst = spool.tile([128, N], f32r, tag="pstage")
                        nc.sync.dma_start(
                            out=pst, in_=pol[k, nt * 128:(nt + 1) * 128, :])
                        nc.gpsimd.tensor_mul(ak[nt], pst, spr[nt])
                    for jt in range(JT):
                        hs = wpool.tile([128, N], mybir.dt.bfloat16, tag=f"hsb{k}_{jt}")
                        hsb[(k, jt)] = hs
                        for mh in range(2):
                            ph = psA.tile([128, 512], f32, tag="pp")
                            for nt in range(NT):
                                nc.tensor.matmul(
                                    ph, xr[nt][:, jt * 128:(jt + 1) * 128],
                                    ak[nt][:, mh * 512:(mh + 1) * 512],
                                    start=(nt == 0), stop=(nt == NT - 1))
                            nc.scalar.copy(
                                hs[:, mh * 512:(mh + 1) * 512], ph)

                # ---- out[t] = relu( sum_k hT_k[t-rows].T @ Theta_k ) ----
                for mh in range(2):
                    for t in range(T):
                        jt, row = divmod(t * F, 128)
                        pg = psG.tile([FC, 512], f32, tag="pg")
                        for k in range(K):
                            nc.tensor.matmul(
                                pg, thr_t[k][row:row + F, :],
                                hsb[(k, jt)][row:row + F, mh * 512:(mh + 1) * 512],
                                start=(k == 0), stop=(k == K - 1),
                                tile_position=(row, 0))
                        gch = spool.tile([FC, 512], f32, tag="gch", bufs=3)
                        nc.scalar.activation(
                            gch, pg, mybir.ActivationFunctionType.Relu)
                        nc.sync.dma_start(
                            out=o_gcn[b, :, t, mh * 512:(mh + 1) * 512], in_=gch)

    nc.compile()
    return nc


def _host_pre(x, node_embeddings, W1, W2, W3, U1, U2, U3, be, Ve):
    """Temporal attention (tiny) + supports, in float64 host math."""
    xd = x
    # E (B,T,T)
    lhs = np.einsum('bnft,n->btf', xd, U1, optimize=True) @ U2
    rhs = np.einsum('f,bnft->bnt', U3, xd, optimize=True)
    prod = np.einsum('btn,bnu->btu', lhs, rhs, optimize=True)
    sig = 1.0 / (1.0 + np.exp(-(prod + be)))
    E = np.einsum('ts,bsu->btu', Ve, sig, optimize=True)
    E = E - E.max(axis=1, keepdims=True)
    E = np.exp(E)
    E = E / E.sum(axis=1, keepdims=True)
    # fold x_TAt away:  l2 = (x . (E @ W1)) @ W2 ;  r2 = E^T @ (W3 . x)
    e1 = E @ W1                                         # (B,T)
    l2a = np.einsum('bnfs,bs->bnf', xd, e1, optimize=True)  # (B,N,F)
    l2 = l2a @ W2                                       # (B,N,T)
    w3x = np.einsum('f,bmfs->bsm', W3, xd, optimize=True)   # (B,T,N)
    r2 = np.einsum('bst,bsm->btm', E, w3x, optimize=True)   # (B,T,N)
    l2T = np.ascontiguousarray(l2.transpose(0, 2, 1))   # (B,T,N)
    # supports
    emb = node_embeddings
    g = emb @ emb.T
    g = np.maximum(g, 0.0)
    g = np.exp(g - g.max(axis=1, keepdims=True))
    sup = g / g.sum(axis=1, keepdims=True)
    eye = np.eye(N, dtype=np.float32)
    polys = np.stack([eye, sup, 2.0 * (sup @ sup) - eye])
    return (np.ascontiguousarray(l2T, dtype=np.float32),
            np.ascontiguousarray(r2, dtype=np.float32),
            np.ascontiguousarray(polys, dtype=np.float32))


def _host_post(x, gcn, tc_v, tc_g, tc_b, rc_w, rc_b, ln_w, ln_b):
    """time conv (weight-normed 1x3) + residual 1x1 conv + layernorm."""
    g = gcn                                             # (B,T,N,FC)
    vnorm = np.sqrt((tc_v.astype(np.float64) ** 2).sum(axis=(1, 2, 3), keepdims=True))
    w = (tc_g[:, None, None, None] * tc_v / vnorm.astype(np.float32))[:, :, 0, :]
    gp = np.pad(g, ((0, 0), (1, 1), (0, 0), (0, 0)))    # pad t -> (B,T+2,N,C)
    # y built directly in (B,T,N,FT): yt[b,t,n,o]
    yt = np.einsum('btnc,oc->btno', gp[:, 0:T], w[:, :, 0], optimize=True)
    yt += np.einsum('btnc,oc->btno', gp[:, 1:T + 1], w[:, :, 1], optimize=True)
    yt += np.einsum('btnc,oc->btno', gp[:, 2:T + 2], w[:, :, 2], optimize=True)
    yt += tc_b[None, None, None, :]
    yt += np.einsum('bnft,of->btno', x, rc_w[:, :, 0, 0], optimize=True)
    yt += rc_b[None, None, None, :]
    mu = yt.mean(axis=-1, keepdims=True)
    var = yt.var(axis=-1, keepdims=True)
    yt = (yt - mu) / np.sqrt(var + LN_EPS) * ln_w + ln_b
    return np.ascontiguousarray(yt.transpose(0, 2, 3, 1)).astype(np.float32)


def kernel(x, node_embeddings, W1, W2, W3, bs, Vs, U1, U2, U3, be, Ve,
           Theta, tc_v, tc_g, tc_b, rc_w, rc_b, ln_w, ln_b):
    x = np.asarray(x, dtype=np.float32)
    l2T, r2, polys = _host_pre(
        x, np.asarray(node_embeddings), np.asarray(W1), np.asarray(W2),
        np.asarray(W3), np.asarray(U1), np.asarray(U2), np.asarray(U3),
        np.asarray(be), np.asarray(Ve))

    xtf = np.ascontiguousarray(x.transpose(0, 1, 3, 2).reshape(B, N, T * F))
    vsT = np.ascontiguousarray(np.asarray(Vs, dtype=np.float32).T)
    bs2 = np.ascontiguousarray(np.asarray(bs, dtype=np.float32)[0])
    theta = np.ascontiguousarray(np.tile(np.asarray(Theta, dtype=np.float32), (1, 4, 1)))

    if "nc" not in _CACHE:
        _CACHE["nc"] = _build()
    nc = _CACHE["nc"]

    in_maps = []
    for c in range(NCORES):
        sl = slice(c * BLOC, (c + 1) * BLOC)
        in_maps.append(dict(
            xtf=xtf[sl], l2T=l2T[sl], r2=r2[sl], bs=bs2, vsT=vsT,
            polys=polys, theta=theta, eye=np.eye(128, dtype=np.float32)))
    res = run_bass_kernel_spmd(nc, in_maps, core_ids=list(range(NCORES)))

    spat = np.concatenate([r["out_spat"] for r in res.results], axis=0)
    gcnT = np.concatenate([r["out_gcn"] for r in res.results], axis=0)
    gcn = gcnT.transpose(0, 2, 3, 1)                    # (B,T,N,FC)

    x_res = _host_post(x, gcn, np.asarray(tc_v), np.asarray(tc_g),
                       np.asarray(tc_b), np.asarray(rc_w), np.asarray(rc_b),
                       np.asarray(ln_w), np.asarray(ln_b))
    return x_res, spat


# revision 12
# speedup vs baseline: 2.2431x; 1.0057x over previous
"""AAST block kernel for 8 TRN2 NeuronCores.

Sharding: data-parallel over batch B=16 -> 2 batches per core, all params
replicated. No collectives. Device computes the dominant FLOPs (spatial
attention chain: prod2 -> sigmoid -> S=Vs@sig -> softmax -> threshold -> spat,
and the K-order graph conv h_k = A_k^T @ x -> Theta -> relu -> gcn) in
float32r matmuls (full-rate fp32-storage matmul mode, ~1e-3 rel precision).
Tiny temporal attention (T=12 matrices), the N x N support polynomials, and
the cheap depthwise time-conv + layernorm epilogue run on host numpy.
"""
import numpy as np

import concourse.bass as bass
import concourse.tile as tile
from concourse import bacc, mybir
from concourse.bass_utils import run_bass_kernel_spmd

B, N, F, T = 16, 1024, 32, 12
K, FC, FT, TS = 3, 64, 64, 1
THETA_THR = 0.6
LN_EPS = 1e-5
NCORES = 8
BLOC = B // NCORES  # 2 batches per core

f32 = mybir.dt.float32
f32r = mybir.dt.float32r

NT = N // 128  # 8 n-tiles
JT = (T * F) // 128  # 3 tiles of the (t,f) axis

_CACHE = {}


def _build():
    nc = bacc.Bacc("TRN2", target_bir_lowering=False, debug=False,
                   num_devices=NCORES)

    # per-core inputs ------------------------------------------------------
    xtf = nc.declare_dram_parameter("xtf", [BLOC, N, T * F], f32r, isOutput=False)
    l2T = nc.declare_dram_parameter("l2T", [BLOC, T, N], f32r, isOutput=False)
    r2d = nc.declare_dram_parameter("r2", [BLOC, T, N], f32r, isOutput=False)
    bsd = nc.declare_dram_parameter("bs", [N, N], f32r, isOutput=False)
    vsT = nc.declare_dram_parameter("vsT", [N, N], f32r, isOutput=False)
    pol = nc.declare_dram_parameter("polys", [K, N, N], f32r, isOutput=False)
    thd = nc.declare_dram_parameter("theta", [K, 128, FC], f32, isOutput=False)
    eyed = nc.declare_dram_parameter("eye", [128, 128], f32r, isOutput=False)
    o_spat = nc.declare_dram_parameter("out_spat", [BLOC, N, N], f32, isOutput=True)
    # gcn transposed: [b, c, t, n]
    o_gcn = nc.declare_dram_parameter("out_gcn", [BLOC, FC, T, N], f32, isOutput=True)
    scr = nc.dram_tensor("thr_scratch", [BLOC, 1], f32)

    with tile.TileContext(nc) as tc:
        with (
            tc.tile_pool(name="const", bufs=1) as cpool,
            tc.tile_pool(name="stage", bufs=2) as spool,
            tc.tile_pool(name="mats", bufs=1) as mpool,
            tc.tile_pool(name="work", bufs=1) as wpool,
            tc.tile_pool(name="psA", bufs=4, space="PSUM") as psA,
            tc.tile_pool(name="psS", bufs=1, space="PSUM") as psS,
            tc.tile_pool(name="psG", bufs=2, space="PSUM") as psG,
        ):
            # ---- constants resident across both batches ----
            ones_f = cpool.tile([128, 1], f32, tag="onesf")
            nc.vector.memset(ones_f, 1.0)
            ones128 = cpool.tile([128, 1], f32r, tag="ones")
            nc.vector.tensor_copy(ones128, ones_f)

            vsr = []  # VsT tiles [128, N] (rows m-tile), f32r direct
            for mt in range(NT):
                t_ = mpool.tile([128, N], f32r, tag=f"vsr{mt}")
                nc.sync.dma_start(out=t_, in_=vsT[mt * 128:(mt + 1) * 128, :])
                vsr.append(t_)
            eyer = cpool.tile([128, 128], f32r, tag="eyer")
            nc.sync.dma_start(out=eyer, in_=eyed[:, :])

            thr_t = []  # Theta rounded, [F, FC] each
            for k in range(K):
                stg = spool.tile([128, FC], f32, tag="thstage")
                nc.sync.dma_start(out=stg, in_=thd[k])
                t_ = cpool.tile([128, FC], mybir.dt.bfloat16, tag=f"theta{k}")
                nc.vector.tensor_copy(t_, stg)
                thr_t.append(t_)

            for b in range(BLOC):
                # ---- load per-batch smalls ----
                l2r = wpool.tile([T, N], f32r, tag="l2r")
                r2r = wpool.tile([T, N], f32r, tag="r2r")
                nc.sync.dma_start(out=l2r, in_=l2T[b])
                nc.sync.dma_start(out=r2r, in_=r2d[b])

                xr = []  # x[b] as [n, (t,f)], f32r direct
                for nt in range(NT):
                    t_ = wpool.tile([128, T * F], f32r, tag=f"xr{nt}")
                    nc.sync.dma_start(out=t_, in_=xtf[b, nt * 128:(nt + 1) * 128, :])
                    xr.append(t_)

                # ---- prod2 -> sigmoid(prod2 + bs) ----
                sig = [wpool.tile([128, N], f32r, tag=f"sig{nt}", name=f"sig{nt}") for nt in range(NT)]
                for nt in range(NT):
                    bst = spool.tile([128, N], f32r, tag="bstage")
                    nc.sync.dma_start(out=bst, in_=bsd[nt * 128:(nt + 1) * 128, :])
                    for mh in range(2):
                        pp = psA.tile([128, 512], f32, tag="pp")
                        nc.tensor.matmul(
                            pp, l2r[:, nt * 128:(nt + 1) * 128],
                            r2r[:, mh * 512:(mh + 1) * 512],
                            start=True, stop=False)
                        nc.tensor.matmul(
                            pp, eyer, bst[:, mh * 512:(mh + 1) * 512],
                            start=False, stop=True)
                        nc.scalar.activation(
                            sig[nt][:, mh * 512:(mh + 1) * 512], pp,
                            mybir.ActivationFunctionType.Sigmoid)

                # ---- S = Vs @ sig  (via lhsT = VsT tiles), exp ----
                expS = [wpool.tile([128, N], f32r, tag=f"expS{nt}", name=f"expS{nt}_{b}") for nt in range(NT)]
                for nt in range(NT):
                    for kh in range(2):
                        ps = psA.tile([128, 512], f32, tag="pp")
                        for mt in range(NT):
                            nc.tensor.matmul(
                                ps, vsr[mt][:, nt * 128:(nt + 1) * 128],
                                sig[mt][:, kh * 512:(kh + 1) * 512],
                                start=(mt == 0), stop=(mt == NT - 1))
                        nc.scalar.activation(
                            expS[nt][:, kh * 512:(kh + 1) * 512], ps,
                            mybir.ActivationFunctionType.Exp)

                # ---- column sums of expS -> reciprocal -> thr ----
                cs = spool.tile([1, N], f32, tag="cs")
                for kh in range(2):
                    pc = psS.tile([1, 512], f32, tag="pcs")
                    for nt in range(NT):
                        nc.tensor.matmul(
                            pc, ones128, expS[nt][:, kh * 512:(kh + 1) * 512],
                            start=(nt == 0), stop=(nt == NT - 1))
                    nc.vector.tensor_copy(cs[:, kh * 512:(kh + 1) * 512], pc)
                rcs = spool.tile([1, N], f32, tag="rcs")
                nc.vector.reciprocal(rcs, cs)
                # total sum of Sn = sum_k cs[k]*rcs[k] ( = N up to fp error)
                prod_cr = spool.tile([1, N], f32, tag="pcr")
                nc.vector.tensor_mul(prod_cr, cs, rcs)
                tot = spool.tile([1, 1], f32, tag="tot")
                nc.vector.reduce_sum(tot, prod_cr, axis=mybir.AxisListType.X)
                thr1 = spool.tile([1, 1], f32, tag="thr1")
                nc.scalar.mul(thr1, tot, 1.0 / (float(N) * float(N) * THETA_THR))
                nc.sync.dma_start(out=scr[b], in_=thr1)
                thr128 = spool.tile([128, 1], f32, tag="thr128")
                nc.sync.dma_start(
                    out=thr128,
                    in_=bass.AP(tensor=scr.ap().tensor,
                                offset=scr.ap().offset + b,
                                ap=[[0, 128], [1, 1]]))
                # broadcast rcs to all partitions via DRAM bounce
                rcs_d = nc.dram_tensor(f"rcs_d{b}", [N], f32)
                nc.sync.dma_start(out=rcs_d[:], in_=rcs)
                rcs_b = spool.tile([128, N], f32, tag="rcsb", bufs=1)
                nc.sync.dma_start(
                    out=rcs_b,
                    in_=bass.AP(tensor=rcs_d.ap().tensor,
                                offset=rcs_d.ap().offset,
                                ap=[[0, 128], [1, N]]))

                # ---- Sn, threshold -> spat (fp32 out + f32r for conv) ----
                spr = expS
                for nt in range(NT):
                    sn = spool.tile([128, N], f32, tag="sn", bufs=2)
                    nc.vector.tensor_mul(sn, expS[nt], rcs_b)
                    mask = spool.tile([128, N], f32, tag="mask", bufs=1)
                    nc.vector.tensor_scalar(
                        mask, sn, thr128, None, op0=mybir.AluOpType.is_ge)
                    nc.vector.tensor_mul(spr[nt], sn, mask)
                    nc.sync.dma_start(
                        out=o_spat[b, nt * 128:(nt + 1) * 128, :],
                        in_=spr[nt].bitcast(f32))

                # ---- graph conv: hT_k = x^T @ (polys_k * spat) ----
                hsb = {}
                for k in range(K):
                    ak = [wpool.tile([128, N], f32r, tag=f"sig{nt}", name=f"ak{nt}_{b}_{k}") for nt in range(NT)]
                    for nt in range(NT):
                        pst = spool.tile([128, N], f32r, tag="pstage", bufs=4)
                        nc.sync.dma_start(
                            out=pst, in_=pol[k, nt * 128:(nt + 1) * 128, :])
                        eng = nc.vector if k == 1 else nc.gpsimd
                        eng.tensor_mul(ak[nt], pst, spr[nt])
                    for jt in range(JT):
                        hs = wpool.tile([128, N], mybir.dt.bfloat16, tag=f"hsb{k}_{jt}")
                        hsb[(k, jt)] = hs
                        for mh in range(2):
                            ph = psA.tile([128, 512], f32, tag="pp")
                            for nt in range(NT):
                                nc.tensor.matmul(
                                    ph, xr[nt][:, jt * 128:(jt + 1) * 128],
                                    ak[nt][:, mh * 512:(mh + 1) * 512],
                                    start=(nt == 0), stop=(nt == NT - 1))
                            nc.vector.tensor_copy(
                                hs[:, mh * 512:(mh + 1) * 512], ph)

                # ---- out[t] = relu( sum_k hT_k[t-rows].T @ Theta_k ) ----
                for mh in range(2):
                    for t in range(T):
                        jt, row = divmod(t * F, 128)
                        pg = psG.tile([FC, 512], f32, tag="pg")
                        for k in range(K):
                            nc.tensor.matmul(
                                pg, thr_t[k][row:row + F, :],
                                hsb[(k, jt)][row:row + F, mh * 512:(mh + 1) * 512],
                                start=(k == 0), stop=(k == K - 1),
                                tile_position=(row, 0))
                        gch = spool.tile([FC, 512], f32, tag="gch", bufs=3)
                        nc.scalar.activation(
                            gch, pg, mybir.ActivationFunctionType.Relu)
                        nc.sync.dma_start(
                            out=o_gcn[b, :, t, mh * 512:(mh + 1) * 512], in_=gch)

    nc.compile()
    return nc


def _host_pre(x, node_embeddings, W1, W2, W3, U1, U2, U3, be, Ve):
    """Temporal attention (tiny) + supports, in float64 host math."""
    xd = x
    # E (B,T,T)
    lhs = np.einsum('bnft,n->btf', xd, U1, optimize=True) @ U2
    rhs = np.einsum('f,bnft->bnt', U3, xd, optimize=True)
    prod = np.einsum('btn,bnu->btu', lhs, rhs, optimize=True)
    sig = 1.0 / (1.0 + np.exp(-(prod + be)))
    E = np.einsum('ts,bsu->btu', Ve, sig, optimize=True)
    E = E - E.max(axis=1, keepdims=True)
    E = np.exp(E)
    E = E / E.sum(axis=1, keepdims=True)
    # fold x_TAt away:  l2 = (x . (E @ W1)) @ W2 ;  r2 = E^T @ (W3 . x)
    e1 = E @ W1                                         # (B,T)
    l2a = np.einsum('bnfs,bs->bnf', xd, e1, optimize=True)  # (B,N,F)
    l2 = l2a @ W2                                       # (B,N,T)
    w3x = np.einsum('f,bmfs->bsm', W3, xd, optimize=True)   # (B,T,N)
    r2 = np.einsum('bst,bsm->btm', E, w3x, optimize=True)   # (B,T,N)
    l2T = np.ascontiguousarray(l2.transpose(0, 2, 1))   # (B,T,N)
    # supports
    emb = node_embeddings
    g = emb @ emb.T
    g = np.maximum(g, 0.0)
    g = np.exp(g - g.max(axis=1, keepdims=True))
    sup = g / g.sum(axis=1, keepdims=True)
    eye = np.eye(N, dtype=np.float32)
    polys = np.stack([eye, sup, 2.0 * (sup @ sup) - eye])
    return (np.ascontiguousarray(l2T, dtype=np.float32),
            np.ascontiguousarray(r2, dtype=np.float32),
            np.ascontiguousarray(polys, dtype=np.float32))


def _host_post(x, gcn, tc_v, tc_g, tc_b, rc_w, rc_b, ln_w, ln_b):
    """time conv (weight-normed 1x3) + residual 1x1 conv + layernorm."""
    g = gcn                                             # (B,T,N,FC)
    vnorm = np.sqrt((tc_v.astype(np.float64) ** 2).sum(axis=(1, 2, 3), keepdims=True))
    w = (tc_g[:, None, None, None] * tc_v / vnorm.astype(np.float32))[:, :, 0, :]
    gp = np.pad(g, ((0, 0), (1, 1), (0, 0), (0, 0)))    # pad t -> (B,T+2,N,C)
    # y built directly in (B,T,N,FT): yt[b,t,n,o]
    yt = np.einsum('btnc,oc->btno', gp[:, 0:T], w[:, :, 0], optimize=True)
    yt += np.einsum('btnc,oc->btno', gp[:, 1:T + 1], w[:, :, 1], optimize=True)
    yt += np.einsum('btnc,oc->btno', gp[:, 2:T + 2], w[:, :, 2], optimize=True)
    yt += tc_b[None, None, None, :]
    yt += np.einsum('bnft,of->btno', x, rc_w[:, :, 0, 0], optimize=True)
    yt += rc_b[None, None, None, :]
    mu = yt.mean(axis=-1, keepdims=True)
    var = yt.var(axis=-1, keepdims=True)
    yt = (yt - mu) / np.sqrt(var + LN_EPS) * ln_w + ln_b
    return np.ascontiguousarray(yt.transpose(0, 2, 3, 1)).astype(np.float32)


def kernel(x, node_embeddings, W1, W2, W3, bs, Vs, U1, U2, U3, be, Ve,
           Theta, tc_v, tc_g, tc_b, rc_w, rc_b, ln_w, ln_b):
    x = np.asarray(x, dtype=np.float32)
    l2T, r2, polys = _host_pre(
        x, np.asarray(node_embeddings), np.asarray(W1), np.asarray(W2),
        np.asarray(W3), np.asarray(U1), np.asarray(U2), np.asarray(U3),
        np.asarray(be), np.asarray(Ve))

    xtf = np.ascontiguousarray(x.transpose(0, 1, 3, 2).reshape(B, N, T * F))
    vsT = np.ascontiguousarray(np.asarray(Vs, dtype=np.float32).T)
    bs2 = np.ascontiguousarray(np.asarray(bs, dtype=np.float32)[0])
    theta = np.ascontiguousarray(np.tile(np.asarray(Theta, dtype=np.float32), (1, 4, 1)))

    if "nc" not in _CACHE:
        _CACHE["nc"] = _build()
    nc = _CACHE["nc"]

    in_maps = []
    for c in range(NCORES):
        sl = slice(c * BLOC, (c + 1) * BLOC)
        in_maps.append(dict(
            xtf=xtf[sl], l2T=l2T[sl], r2=r2[sl], bs=bs2, vsT=vsT,
            polys=polys, theta=theta, eye=np.eye(128, dtype=np.float32)))
    res = run_bass_kernel_spmd(nc, in_maps, core_ids=list(range(NCORES)))

    spat = np.concatenate([r["out_spat"] for r in res.results], axis=0)
    gcnT = np.concatenate([r["out_gcn"] for r in res.results], axis=0)
    gcn = gcnT.transpose(0, 2, 3, 1)                    # (B,T,N,FC)

    x_res = _host_post(x, gcn, np.asarray(tc_v), np.asarray(tc_g),
                       np.asarray(tc_b), np.asarray(rc_w), np.asarray(rc_b),
                       np.asarray(ln_w), np.asarray(ln_b))
    return x_res, spat
